# revision 1
# baseline (speedup 1.0000x reference)
"""SchNet InteractionBlock on 8 trn2 NeuronCores (Bass/Tile).

Strategy: sort edges by dst on host; core k owns nodes [k*6250,(k+1)*6250)
and exactly the edges targeting them -> no all-reduce needed.
Scatter-add is a one-hot selection-matrix matmul accumulated per
128-node block in PSUM. Filter MLP runs in feature-on-partition layout
with constant stationary weights. x@w1.T is precomputed per core into a
bf16 DRAM table and rows are fetched per edge tile via indirect DMA.

ssp(x) = softplus(x)-log2 is computed as relu(x) + p(min(e^x, e^-x))
with p a deg-2 minimax fit of log1p(u)-log2 on [0,1] (|err| < 8.2e-3),
using only Exp/Relu/Copy from the single `exp_and_friends` ACT table.
Biases are injected as K=2 rank-1 matmuls (hi/lo bf16 split) into PSUM.
"""

import numpy as np
import ml_dtypes

import concourse.bacc as bacc
import concourse.bass as bass
import concourse.mybir as mybir
import concourse.tile as tile
from concourse.bass_utils import run_bass_kernel_spmd

N = 50000
E = 600000
HID = 128
NF = 128
NG = 50
CUTOFF = 10.0
NCORES = 8
NPC = N // NCORES          # 6250 nodes per core
NBLK = (NPC + 127) // 128  # 49 blocks (last one has 106 nodes)
P = 128

BF16 = mybir.dt.bfloat16
F32 = mybir.dt.float32
I32 = mybir.dt.int32
AF = mybir.ActivationFunctionType
OP = mybir.AluOpType
LOG2 = float(np.log(2.0))
BF = ml_dtypes.bfloat16

# deg-2 minimax of log1p(u) on [0,1], with -log2 folded into C0
C2 = -0.22253306
C1 = 0.90520375
C0 = 0.00818788 - LOG2

LAST_RESULT = None  # BassKernelResults of the most recent run (for test harness)


def _hilo(v):
    hi = v.astype(BF)
    lo = (v - hi.astype(np.float32)).astype(BF)
    return np.ascontiguousarray(np.stack([hi, lo]))


def _build_nc(TT, blk_start, blk_end, block_of_tile):
    EP = TT * P
    nc = bacc.Bacc()

    xT_d = nc.dram_tensor("xT", [HID, N], BF16, kind="ExternalInput")
    basisT_d = nc.dram_tensor("basisT", [NG + 1, EP], BF16, kind="ExternalInput")
    srcI_d = nc.dram_tensor("srcI", [P, TT], I32, kind="ExternalInput")
    dstl_d = nc.dram_tensor("dstl", [P, TT], BF16, kind="ExternalInput")
    cmul_d = nc.dram_tensor("cmul", [P, TT], F32, kind="ExternalInput")
    fw1T_d = nc.dram_tensor("fw1T", [NG + 1, NF], BF16, kind="ExternalInput")
    fw2T_d = nc.dram_tensor("fw2T", [NF, NF], BF16, kind="ExternalInput")
    fb2two_d = nc.dram_tensor("fb2two", [2, NF], BF16, kind="ExternalInput")
    w1T_d = nc.dram_tensor("w1T", [HID, NF], BF16, kind="ExternalInput")
    w2T_d = nc.dram_tensor("w2T", [NF, HID], BF16, kind="ExternalInput")
    b2two_d = nc.dram_tensor("b2two", [2, HID], BF16, kind="ExternalInput")
    w3T_d = nc.dram_tensor("w3T", [HID, HID], BF16, kind="ExternalInput")
    b3two_d = nc.dram_tensor("b3two", [2, HID], BF16, kind="ExternalInput")
    ones2_d = nc.dram_tensor("ones2", [2, P], BF16, kind="ExternalInput")
    iota_d = nc.dram_tensor("iota", [P, P], BF16, kind="ExternalInput")
    outT_d = nc.dram_tensor("outT", [HID, NPC], F32, kind="ExternalOutput")

    with tile.TileContext(nc) as tc:
        with (
            tc.tile_pool(name="const", bufs=1) as cp,
            tc.tile_pool(name="dram", bufs=1, space="DRAM") as dp,
            tc.tile_pool(name="arr", bufs=1) as arp,
            tc.tile_pool(name="bchunk", bufs=2) as bp,
            tc.tile_pool(name="xtp", bufs=2) as xp,
            tc.tile_pool(name="work", bufs=3) as wp,
            tc.tile_pool(name="hsp", bufs=2) as hp,
            tc.tile_pool(name="psA", bufs=2, space="PSUM") as psA,
            tc.tile_pool(name="psB", bufs=2, space="PSUM") as psB,
            tc.tile_pool(name="psC", bufs=2, space="PSUM") as psC,
            tc.tile_pool(name="psD", bufs=2, space="PSUM") as psD,
        ):
            def cload(dram, shape, dtype):
                t = cp.tile(shape, dtype, tag=dram.name)
                nc.sync.dma_start(out=t[:], in_=dram[:])
                return t

            xh_d = dp.tile([N, NF], BF16, tag="xh_tab")
            fw1T = cload(fw1T_d, [NG + 1, NF], BF16)
            fw2T = cload(fw2T_d, [NF, NF], BF16)
            fb2two = cload(fb2two_d, [2, NF], BF16)
            w1T = cload(w1T_d, [HID, NF], BF16)
            w2T = cload(w2T_d, [NF, HID], BF16)
            b2two = cload(b2two_d, [2, HID], BF16)
            w3T = cload(w3T_d, [HID, HID], BF16)
            b3two = cload(b3two_d, [2, HID], BF16)
            ones2 = cload(ones2_d, [2, P], BF16)
            iota = cload(iota_d, [P, P], BF16)

            srcI = arp.tile([P, TT], I32, tag="srcI")
            nc.sync.dma_start(out=srcI[:], in_=srcI_d[:])
            dstl = arp.tile([P, TT], BF16, tag="dstl")
            nc.sync.dma_start(out=dstl[:], in_=dstl_d[:])
            cmul = arp.tile([P, TT], F32, tag="cmul")
            nc.sync.dma_start(out=cmul[:], in_=cmul_d[:])
            outT = arp.tile([HID, NPC], F32, tag="outT")

            # ---- phase 1: xh table = x @ w1.T, stored bf16 [N, NF] ----
            CH = 2048
            for c0 in range(0, N, CH):
                c1 = min(c0 + CH, N)
                w = c1 - c0
                xt = xp.tile([P, CH], BF16, tag="xt")
                nc.sync.dma_start(out=xt[:, :w], in_=xT_d[:, c0:c1])
                xo = xp.tile([P, CH], BF16, tag="xo")
                nt = (w + P - 1) // P
                for j in range(nt):
                    n0 = j * P
                    n1 = min(n0 + P, w)
                    m = n1 - n0
                    ps = psB.tile([P, P], F32, tag="psB")
                    nc.tensor.matmul(out=ps[:m, :], lhsT=xt[:, n0:n1],
                                     rhs=w1T[:], start=True, stop=True)
                    nc.vector.tensor_copy(out=xo[:m, j * P:j * P + P], in_=ps[:m, :])
                for j in range(nt):
                    n0 = j * P
                    n1 = min(n0 + P, w)
                    m = n1 - n0
                    nc.sync.dma_start(out=xh_d[c0 + n0:c0 + n1, :],
                                      in_=xo[:m, j * P:j * P + P])

            # ---- phase 2: edges ----
            BT = 64  # tiles per basis chunk
            bch = None
            hsT = None
            agg = None
            for t in range(TT):
                chn, s = divmod(t, BT)
                if s == 0:
                    bch = bp.tile([NG + 1, BT * P], BF16, tag="bch")
                    w = min(BT * P, EP - chn * BT * P)
                    nc.sync.dma_start(out=bch[:, :w],
                                      in_=basisT_d[:, chn * BT * P:chn * BT * P + w])
                if t % 4 == 0:
                    h1 = psA.tile([P, 512], F32, tag="h1")
                    nc.tensor.matmul(out=h1[:], lhsT=fw1T[:],
                                     rhs=bch[:, s * P:(s + 4) * P],
                                     start=True, stop=True)
                    # ssp(x) = relu(x) + p(min(e^x, e^-x))
                    e1 = hp.tile([P, 512], F32, tag="e1")
                    nc.scalar.activation(e1[:], h1[:], AF.Exp, scale=-1.0)
                    e2 = hp.tile([P, 512], F32, tag="e2")
                    nc.scalar.activation(e2[:], h1[:], AF.Exp, scale=1.0)
                    rl = hp.tile([P, 512], F32, tag="rl")
                    nc.scalar.activation(rl[:], h1[:], AF.Relu)
                    u = hp.tile([P, 512], F32, tag="u")
                    nc.vector.tensor_tensor(out=u[:], in0=e1[:], in1=e2[:], op=OP.min)
                    q = hp.tile([P, 512], F32, tag="q")
                    nc.vector.tensor_scalar(q[:], u[:], C2, C1, OP.mult, OP.add)
                    tq = hp.tile([P, 512], F32, tag="tq")
                    nc.vector.tensor_mul(out=tq[:], in0=q[:], in1=u[:])
                    hsT = hp.tile([P, 512], BF16, tag="hsT")
                    nc.vector.affine_then_add(hsT[:], tq[:], rl[:], 1.0, C0)
                q = t % 4
                wq = psB.tile([P, P], F32, tag="psB")
                nc.tensor.matmul(out=wq[:], lhsT=hsT[:, q * P:(q + 1) * P],
                                 rhs=fw2T[:], start=True, stop=False)
                nc.tensor.matmul(out=wq[:], lhsT=ones2[:], rhs=fb2two[:],
                                 start=False, stop=True, skip_group_check=True)
                xg = wp.tile([P, NF], BF16, tag="xg")
                nc.gpsimd.indirect_dma_start(
                    out=xg[:], out_offset=None, in_=xh_d[:],
                    in_offset=bass.IndirectOffsetOnAxis(ap=srcI[:, t:t + 1], axis=0),
                )
                # msg = (wq * C) * xh_gather
                msg = wp.tile([P, NF], BF16, tag="msg")
                nc.vector.scalar_tensor_tensor(out=msg[:], in0=wq[:],
                                               scalar=cmul[:, t:t + 1], in1=xg[:],
                                               op0=OP.mult, op1=OP.mult)
                S = wp.tile([P, P], BF16, tag="S")
                nc.vector.tensor_tensor(out=S[:],
                                        in0=dstl[:, t:t + 1].to_broadcast([P, P]),
                                        in1=iota[:], op=OP.is_equal)
                b = block_of_tile[t]
                if t == blk_start[b]:
                    agg = psC.tile([P, P], F32, tag="agg")
                nc.tensor.matmul(out=agg[:], lhsT=msg[:], rhs=S[:],
                                 start=(t == blk_start[b]), stop=(t == blk_end[b]),
                                 skip_group_check=True)
                if t == blk_end[b]:
                    nb = min(P, NPC - b * P)
                    aggs = wp.tile([P, P], BF16, tag="aggs")
                    nc.scalar.copy(out=aggs[:], in_=agg[:])
                    z1 = psD.tile([P, P], F32, tag="z")
                    nc.tensor.matmul(out=z1[:, :nb], lhsT=w2T[:],
                                     rhs=aggs[:, :nb], start=True, stop=False)
                    nc.tensor.matmul(out=z1[:, :nb], lhsT=b2two[:],
                                     rhs=ones2[:, :nb], start=False, stop=True,
                                     skip_group_check=True)
                    e1z = wp.tile([P, P], F32, tag="e1z")
                    nc.scalar.activation(e1z[:, :nb], z1[:, :nb], AF.Exp, scale=-1.0)
                    e2z = wp.tile([P, P], F32, tag="e2z")
                    nc.scalar.activation(e2z[:, :nb], z1[:, :nb], AF.Exp, scale=1.0)
                    rlz = wp.tile([P, P], F32, tag="rlz")
                    nc.scalar.activation(rlz[:, :nb], z1[:, :nb], AF.Relu)
                    uz = wp.tile([P, P], F32, tag="uz")
                    nc.vector.tensor_tensor(out=uz[:, :nb], in0=e1z[:, :nb],
                                            in1=e2z[:, :nb], op=OP.min)
                    qz = wp.tile([P, P], F32, tag="qz")
                    nc.vector.tensor_scalar(qz[:, :nb], uz[:, :nb], C2, C1,
                                            OP.mult, OP.add)
                    tqz = wp.tile([P, P], F32, tag="tqz")
                    nc.vector.tensor_mul(out=tqz[:, :nb], in0=qz[:, :nb],
                                         in1=uz[:, :nb])
                    z1s = wp.tile([P, P], BF16, tag="z1s")
                    nc.vector.affine_then_add(z1s[:, :nb], tqz[:, :nb], rlz[:, :nb],
                                              1.0, C0)
                    z2 = psD.tile([P, P], F32, tag="z")
                    nc.tensor.matmul(out=z2[:, :nb], lhsT=w3T[:],
                                     rhs=z1s[:, :nb], start=True, stop=False)
                    nc.tensor.matmul(out=z2[:, :nb], lhsT=b3two[:],
                                     rhs=ones2[:, :nb], start=False, stop=True,
                                     skip_group_check=True)
                    nc.scalar.copy(out=outT[:, b * P:b * P + nb], in_=z2[:, :nb])

            nc.sync.dma_start(out=outT_d[:], in_=outT[:])

    nc.compile()
    return nc


def kernel(**inputs):
    global LAST_RESULT
    x = np.asarray(inputs["x"], np.float32)
    ji = np.asarray(inputs["ji_pairs"])
    e_ji = np.asarray(inputs["e_ji"], np.float32)
    basis = np.asarray(inputs["e_ji_basis"], np.float32)
    fw1 = np.asarray(inputs["fw1"], np.float32)
    fb1 = np.asarray(inputs["fb1"], np.float32)
    fw2 = np.asarray(inputs["fw2"], np.float32)
    fb2 = np.asarray(inputs["fb2"], np.float32)
    w1 = np.asarray(inputs["w1"], np.float32)
    w2 = np.asarray(inputs["w2"], np.float32)
    b2 = np.asarray(inputs["b2"], np.float32)
    w3 = np.asarray(inputs["w3"], np.float32)
    b3 = np.asarray(inputs["b3"], np.float32)

    src = ji[0].astype(np.int64)
    dst = ji[1].astype(np.int64)
    order = np.argsort(dst, kind="stable")
    dsts = dst[order]
    srcs = src[order].astype(np.int32)
    Cs = (0.25 * (np.cos(e_ji * (np.pi / CUTOFF)) + 1.0)).astype(np.float32)[order]
    basis_s = basis[order]

    # per (core, block) edge ranges
    blk_bounds = []
    for k in range(NCORES):
        marks = k * NPC + np.minimum(np.arange(NBLK + 1) * 128, NPC)
        blk_bounds.append(np.searchsorted(dsts, marks))
    cnt = np.array([bb[1:] - bb[:-1] for bb in blk_bounds])  # [NCORES, NBLK]
    T = np.maximum(1, -(-cnt // P)).max(axis=0)              # tiles per block
    if T.sum() % 4:
        T[-1] += 4 - T.sum() % 4
    TT = int(T.sum())
    EP = TT * P
    tile_ofs = np.concatenate([[0], np.cumsum(T)])
    blk_start = [int(tile_ofs[b]) for b in range(NBLK)]
    blk_end = [int(tile_ofs[b + 1] - 1) for b in range(NBLK)]
    block_of_tile = np.repeat(np.arange(NBLK), T)

    srcp = np.zeros((NCORES, EP), np.int32)
    dstlp = np.full((NCORES, EP), -1.0, BF)
    cmp_ = np.zeros((NCORES, EP), np.float32)
    basp = np.zeros((NCORES, NG + 1, EP), BF)
    for k in range(NCORES):
        bb = blk_bounds[k]
        for b in range(NBLK):
            e0, e1 = int(bb[b]), int(bb[b + 1])
            n = e1 - e0
            o = blk_start[b] * P
            srcp[k, o:o + n] = srcs[e0:e1]
            dstlp[k, o:o + n] = (dsts[e0:e1] - (k * NPC + b * 128)).astype(BF)
            cmp_[k, o:o + n] = Cs[e0:e1]
            basp[k, :NG, o:o + n] = basis_s[e0:e1].T.astype(BF)
            basp[k, NG, o:o + n] = np.float32(1.0)

    def col(a):  # [EP] -> [P, TT] with [p,t] = a[t*P+p]
        return np.ascontiguousarray(a.reshape(TT, P).T)

    xT = np.ascontiguousarray(x.T).astype(BF)
    fw1T = np.concatenate([fw1.T, fb1[None, :]], axis=0).astype(BF)
    fw2T = np.ascontiguousarray(fw2.T).astype(BF)
    fb2two = _hilo(fb2)
    w1T = np.ascontiguousarray(w1.T).astype(BF)
    w2T = np.ascontiguousarray(w2.T).astype(BF)
    b2two = _hilo(b2)
    w3T = np.ascontiguousarray(w3.T).astype(BF)
    b3two = _hilo(b3)
    ones2 = np.ones((2, P), BF)
    iota = np.tile(np.arange(P, dtype=np.float32)[None, :], (P, 1)).astype(BF)

    nc = _build_nc(TT, blk_start, blk_end, block_of_tile)

    in_maps = []
    for k in range(NCORES):
        in_maps.append({
            "xT": xT, "basisT": np.ascontiguousarray(basp[k]),
            "srcI": col(srcp[k]), "dstl": col(dstlp[k]), "cmul": col(cmp_[k]),
            "fw1T": fw1T, "fw2T": fw2T, "fb2two": fb2two, "w1T": w1T,
            "w2T": w2T, "b2two": b2two, "w3T": w3T, "b3two": b3two,
            "ones2": ones2, "iota": np.ascontiguousarray(iota),
        })
    res = run_bass_kernel_spmd(nc, in_maps, core_ids=list(range(NCORES)))
    LAST_RESULT = res

    out = np.empty((N, HID), np.float32)
    for k in range(NCORES):
        out[k * NPC:(k + 1) * NPC, :] = res.results[k]["outT"].T
    return out



# revision 2
# speedup vs baseline: 1.4196x; 1.4196x over previous
"""SchNet InteractionBlock on 8 trn2 NeuronCores (Bass/Tile).

Strategy: sort edges by dst on host; core k owns nodes [k*6250,(k+1)*6250)
and exactly the edges targeting them -> no all-reduce needed.
Scatter-add is a one-hot selection-matrix matmul accumulated per
128-node block in PSUM. Filter MLP runs in feature-on-partition layout
with constant stationary weights. x@w1.T is precomputed per core into a
bf16 DRAM table and rows are fetched per edge tile via indirect DMA.

ssp(x) = softplus(x)-log2 is computed as relu(x) + p(min(e^x, e^-x))
with p a deg-2 minimax fit of log1p(u)-log2 on [0,1] (|err| < 8.2e-3),
using only Exp/Relu/Copy from the single `exp_and_friends` ACT table.
Biases are injected as K=2 rank-1 matmuls (hi/lo bf16 split) into PSUM.
"""

import numpy as np
import ml_dtypes

import concourse.bacc as bacc
import concourse.bass as bass
import concourse.mybir as mybir
import concourse.tile as tile
from concourse.bass_utils import run_bass_kernel_spmd

N = 50000
E = 600000
HID = 128
NF = 128
NG = 50
CUTOFF = 10.0
NCORES = 8
NPC = N // NCORES          # 6250 nodes per core
NBLK = (NPC + 127) // 128  # 49 blocks (last one has 106 nodes)
P = 128

BF16 = mybir.dt.bfloat16
F32 = mybir.dt.float32
I32 = mybir.dt.int32
AF = mybir.ActivationFunctionType
OP = mybir.AluOpType
LOG2 = float(np.log(2.0))
BF = ml_dtypes.bfloat16

# deg-2 minimax of log1p(u) on [0,1], with -log2 folded into C0
C2 = -0.22253306
C1 = 0.90520375
C0 = 0.00818788 - LOG2

LAST_RESULT = None  # BassKernelResults of the most recent run (for test harness)


def _hilo(v):
    hi = v.astype(BF)
    lo = (v - hi.astype(np.float32)).astype(BF)
    return np.ascontiguousarray(np.stack([hi, lo]))


def _build_nc(TT, blk_start, blk_end, block_of_tile):
    EP = TT * P
    nc = bacc.Bacc()

    xT_d = nc.dram_tensor("xT", [HID, N], BF16, kind="ExternalInput")
    basisT_d = nc.dram_tensor("basisT", [NG + 1, EP], BF16, kind="ExternalInput")
    srcI_d = nc.dram_tensor("srcI", [P, TT], I32, kind="ExternalInput")
    dstl_d = nc.dram_tensor("dstl", [P, TT], BF16, kind="ExternalInput")
    cmul_d = nc.dram_tensor("cmul", [P, TT], F32, kind="ExternalInput")
    fw1T_d = nc.dram_tensor("fw1T", [NG + 1, NF], BF16, kind="ExternalInput")
    fw2T_d = nc.dram_tensor("fw2T", [NF, NF], BF16, kind="ExternalInput")
    fb2two_d = nc.dram_tensor("fb2two", [2, NF], BF16, kind="ExternalInput")
    w1T_d = nc.dram_tensor("w1T", [HID, NF], BF16, kind="ExternalInput")
    w2T_d = nc.dram_tensor("w2T", [NF, HID], BF16, kind="ExternalInput")
    b2two_d = nc.dram_tensor("b2two", [2, HID], BF16, kind="ExternalInput")
    w3T_d = nc.dram_tensor("w3T", [HID, HID], BF16, kind="ExternalInput")
    b3two_d = nc.dram_tensor("b3two", [2, HID], BF16, kind="ExternalInput")
    ones2_d = nc.dram_tensor("ones2", [2, P], BF16, kind="ExternalInput")
    iota_d = nc.dram_tensor("iota", [P, P], BF16, kind="ExternalInput")
    outT_d = nc.dram_tensor("outT", [HID, NPC], F32, kind="ExternalOutput")

    with tile.TileContext(nc) as tc:
        with (
            tc.tile_pool(name="const", bufs=1) as cp,
            tc.tile_pool(name="dram", bufs=1, space="DRAM") as dp,
            tc.tile_pool(name="arr", bufs=1) as arp,
            tc.tile_pool(name="bchunk", bufs=2) as bp,
            tc.tile_pool(name="xtp", bufs=2) as xp,
            tc.tile_pool(name="work", bufs=3) as wp,
            tc.tile_pool(name="hsp", bufs=2) as hp,
            tc.tile_pool(name="psA", bufs=2, space="PSUM") as psA,
            tc.tile_pool(name="psB", bufs=2, space="PSUM") as psB,
            tc.tile_pool(name="psC", bufs=2, space="PSUM") as psC,
            tc.tile_pool(name="psD", bufs=2, space="PSUM") as psD,
        ):
            def cload(dram, shape, dtype):
                t = cp.tile(shape, dtype, tag=dram.name)
                nc.sync.dma_start(out=t[:], in_=dram[:])
                return t

            xh_d = dp.tile([N, NF], BF16, tag="xh_tab")
            fw1T = cload(fw1T_d, [NG + 1, NF], BF16)
            fw2T = cload(fw2T_d, [NF, NF], BF16)
            fb2two = cload(fb2two_d, [2, NF], BF16)
            w1T = cload(w1T_d, [HID, NF], BF16)
            w2T = cload(w2T_d, [NF, HID], BF16)
            b2two = cload(b2two_d, [2, HID], BF16)
            w3T = cload(w3T_d, [HID, HID], BF16)
            b3two = cload(b3two_d, [2, HID], BF16)
            ones2 = cload(ones2_d, [2, P], BF16)
            iota = cload(iota_d, [P, P], BF16)

            srcI = arp.tile([P, TT], I32, tag="srcI")
            nc.sync.dma_start(out=srcI[:], in_=srcI_d[:])
            dstl = arp.tile([P, TT], BF16, tag="dstl")
            nc.sync.dma_start(out=dstl[:], in_=dstl_d[:])
            cmul = arp.tile([P, TT], F32, tag="cmul")
            nc.sync.dma_start(out=cmul[:], in_=cmul_d[:])
            outT = arp.tile([HID, NPC], F32, tag="outT")

            # ---- phase 1: xh table = x @ w1.T, stored bf16 [N, NF] ----
            CH = 2048
            for c0 in range(0, N, CH):
                c1 = min(c0 + CH, N)
                w = c1 - c0
                xt = xp.tile([P, CH], BF16, tag="xt")
                nc.sync.dma_start(out=xt[:, :w], in_=xT_d[:, c0:c1])
                xo = xp.tile([P, CH], BF16, tag="xo")
                nt = (w + P - 1) // P
                for j in range(nt):
                    n0 = j * P
                    n1 = min(n0 + P, w)
                    m = n1 - n0
                    ps = psB.tile([P, P], F32, tag="psB")
                    nc.tensor.matmul(out=ps[:m, :], lhsT=xt[:, n0:n1],
                                     rhs=w1T[:], start=True, stop=True)
                    nc.vector.tensor_copy(out=xo[:m, j * P:j * P + P], in_=ps[:m, :])
                for j in range(nt):
                    n0 = j * P
                    n1 = min(n0 + P, w)
                    m = n1 - n0
                    nc.sync.dma_start(out=xh_d[c0 + n0:c0 + n1, :],
                                      in_=xo[:m, j * P:j * P + P])

            # ---- phase 2: edges ----
            BT = 64  # tiles per basis chunk
            bch = None
            hsT = None
            agg = None
            for t in range(TT):
                chn, s = divmod(t, BT)
                if s == 0:
                    bch = bp.tile([NG + 1, BT * P], BF16, tag="bch")
                    w = min(BT * P, EP - chn * BT * P)
                    nc.sync.dma_start(out=bch[:, :w],
                                      in_=basisT_d[:, chn * BT * P:chn * BT * P + w])
                if t % 4 == 0:
                    h1 = psA.tile([P, 512], F32, tag="h1")
                    nc.tensor.matmul(out=h1[:], lhsT=fw1T[:],
                                     rhs=bch[:, s * P:(s + 4) * P],
                                     start=True, stop=True)
                    # ssp(x) = relu(x) + p(min(e^x, e^-x))
                    e1 = hp.tile([P, 512], F32, tag="e1")
                    nc.scalar.activation(e1[:], h1[:], AF.Exp, scale=-1.0)
                    e2 = hp.tile([P, 512], F32, tag="e2")
                    nc.scalar.activation(e2[:], h1[:], AF.Exp, scale=1.0)
                    rl = hp.tile([P, 512], F32, tag="rl")
                    nc.scalar.activation(rl[:], h1[:], AF.Relu)
                    u = hp.tile([P, 512], F32, tag="u")
                    nc.vector.tensor_tensor(out=u[:], in0=e1[:], in1=e2[:], op=OP.min)
                    q = hp.tile([P, 512], F32, tag="q")
                    nc.vector.tensor_scalar(q[:], u[:], C2, C1, OP.mult, OP.add)
                    tq = hp.tile([P, 512], F32, tag="tq")
                    nc.vector.tensor_mul(out=tq[:], in0=q[:], in1=u[:])
                    hsT = hp.tile([P, 512], BF16, tag="hsT")
                    nc.vector.affine_then_add(hsT[:], tq[:], rl[:], 1.0, C0)
                q = t % 4
                wq = psB.tile([P, P], F32, tag="psB")
                nc.tensor.matmul(out=wq[:], lhsT=hsT[:, q * P:(q + 1) * P],
                                 rhs=fw2T[:], start=True, stop=False)
                nc.tensor.matmul(out=wq[:], lhsT=ones2[:], rhs=fb2two[:],
                                 start=False, stop=True, skip_group_check=True)
                xg = wp.tile([P, NF], BF16, tag="xg")
                nc.gpsimd.indirect_dma_start(
                    out=xg[:], out_offset=None, in_=xh_d[:],
                    in_offset=bass.IndirectOffsetOnAxis(ap=srcI[:, t:t + 1], axis=0),
                )
                # msg = (wq * C) * xh_gather
                msg = wp.tile([P, NF], BF16, tag="msg")
                nc.vector.scalar_tensor_tensor(out=msg[:], in0=wq[:],
                                               scalar=cmul[:, t:t + 1], in1=xg[:],
                                               op0=OP.mult, op1=OP.mult)
                S = wp.tile([P, P], BF16, tag="S")
                nc.vector.tensor_tensor(out=S[:],
                                        in0=dstl[:, t:t + 1].to_broadcast([P, P]),
                                        in1=iota[:], op=OP.is_equal)
                b = block_of_tile[t]
                if t == blk_start[b]:
                    agg = psC.tile([P, P], F32, tag="agg")
                nc.tensor.matmul(out=agg[:], lhsT=msg[:], rhs=S[:],
                                 start=(t == blk_start[b]), stop=(t == blk_end[b]),
                                 skip_group_check=True)
                if t == blk_end[b]:
                    nb = min(P, NPC - b * P)
                    aggs = wp.tile([P, P], BF16, tag="aggs")
                    nc.scalar.copy(out=aggs[:], in_=agg[:])
                    z1 = psD.tile([P, P], F32, tag="z")
                    nc.tensor.matmul(out=z1[:, :nb], lhsT=w2T[:],
                                     rhs=aggs[:, :nb], start=True, stop=False)
                    nc.tensor.matmul(out=z1[:, :nb], lhsT=b2two[:],
                                     rhs=ones2[:, :nb], start=False, stop=True,
                                     skip_group_check=True)
                    e1z = wp.tile([P, P], F32, tag="e1z")
                    nc.scalar.activation(e1z[:, :nb], z1[:, :nb], AF.Exp, scale=-1.0)
                    e2z = wp.tile([P, P], F32, tag="e2z")
                    nc.scalar.activation(e2z[:, :nb], z1[:, :nb], AF.Exp, scale=1.0)
                    rlz = wp.tile([P, P], F32, tag="rlz")
                    nc.scalar.activation(rlz[:, :nb], z1[:, :nb], AF.Relu)
                    uz = wp.tile([P, P], F32, tag="uz")
                    nc.vector.tensor_tensor(out=uz[:, :nb], in0=e1z[:, :nb],
                                            in1=e2z[:, :nb], op=OP.min)
                    qz = wp.tile([P, P], F32, tag="qz")
                    nc.vector.tensor_scalar(qz[:, :nb], uz[:, :nb], C2, C1,
                                            OP.mult, OP.add)
                    tqz = wp.tile([P, P], F32, tag="tqz")
                    nc.vector.tensor_mul(out=tqz[:, :nb], in0=qz[:, :nb],
                                         in1=uz[:, :nb])
                    z1s = wp.tile([P, P], BF16, tag="z1s")
                    nc.vector.affine_then_add(z1s[:, :nb], tqz[:, :nb], rlz[:, :nb],
                                              1.0, C0)
                    z2 = psD.tile([P, P], F32, tag="z")
                    nc.tensor.matmul(out=z2[:, :nb], lhsT=w3T[:],
                                     rhs=z1s[:, :nb], start=True, stop=False)
                    nc.tensor.matmul(out=z2[:, :nb], lhsT=b3two[:],
                                     rhs=ones2[:, :nb], start=False, stop=True,
                                     skip_group_check=True)
                    nc.scalar.copy(out=outT[:, b * P:b * P + nb], in_=z2[:, :nb])

            nc.sync.dma_start(out=outT_d[:], in_=outT[:])

    nc.compile()
    return nc


def build_for_sim(**inputs):
    """Build the Bass module for core 0 without executing (local sim use)."""
    ji = np.asarray(inputs["ji_pairs"])
    dst = ji[1].astype(np.int64)
    order = np.argsort(dst, kind="stable")
    dsts = dst[order]
    blk_bounds = []
    for k in range(NCORES):
        marks = k * NPC + np.minimum(np.arange(NBLK + 1) * 128, NPC)
        blk_bounds.append(np.searchsorted(dsts, marks))
    cnt = np.array([bb[1:] - bb[:-1] for bb in blk_bounds])
    T = np.maximum(1, -(-cnt // P)).max(axis=0)
    if T.sum() % 4:
        T[-1] += 4 - T.sum() % 4
    TT = int(T.sum())
    tile_ofs = np.concatenate([[0], np.cumsum(T)])
    blk_start = [int(tile_ofs[b]) for b in range(NBLK)]
    blk_end = [int(tile_ofs[b + 1] - 1) for b in range(NBLK)]
    block_of_tile = np.repeat(np.arange(NBLK), T)
    return _build_nc(TT, blk_start, blk_end, block_of_tile)


def kernel(**inputs):
    global LAST_RESULT
    x = np.asarray(inputs["x"], np.float32)
    ji = np.asarray(inputs["ji_pairs"])
    e_ji = np.asarray(inputs["e_ji"], np.float32)
    basis = np.asarray(inputs["e_ji_basis"], np.float32)
    fw1 = np.asarray(inputs["fw1"], np.float32)
    fb1 = np.asarray(inputs["fb1"], np.float32)
    fw2 = np.asarray(inputs["fw2"], np.float32)
    fb2 = np.asarray(inputs["fb2"], np.float32)
    w1 = np.asarray(inputs["w1"], np.float32)
    w2 = np.asarray(inputs["w2"], np.float32)
    b2 = np.asarray(inputs["b2"], np.float32)
    w3 = np.asarray(inputs["w3"], np.float32)
    b3 = np.asarray(inputs["b3"], np.float32)

    src = ji[0].astype(np.int64)
    dst = ji[1].astype(np.int64)
    order = np.argsort(dst, kind="stable")
    dsts = dst[order]
    srcs = src[order].astype(np.int32)
    Cs = (0.25 * (np.cos(e_ji * (np.pi / CUTOFF)) + 1.0)).astype(np.float32)[order]
    basis_s = basis[order]

    # per (core, block) edge ranges
    blk_bounds = []
    for k in range(NCORES):
        marks = k * NPC + np.minimum(np.arange(NBLK + 1) * 128, NPC)
        blk_bounds.append(np.searchsorted(dsts, marks))
    cnt = np.array([bb[1:] - bb[:-1] for bb in blk_bounds])  # [NCORES, NBLK]
    T = np.maximum(1, -(-cnt // P)).max(axis=0)              # tiles per block
    if T.sum() % 4:
        T[-1] += 4 - T.sum() % 4
    TT = int(T.sum())
    EP = TT * P
    tile_ofs = np.concatenate([[0], np.cumsum(T)])
    blk_start = [int(tile_ofs[b]) for b in range(NBLK)]
    blk_end = [int(tile_ofs[b + 1] - 1) for b in range(NBLK)]
    block_of_tile = np.repeat(np.arange(NBLK), T)

    srcp = np.zeros((NCORES, EP), np.int32)
    dstlp = np.full((NCORES, EP), -1.0, BF)
    cmp_ = np.zeros((NCORES, EP), np.float32)
    basp = np.zeros((NCORES, NG + 1, EP), BF)
    for k in range(NCORES):
        bb = blk_bounds[k]
        for b in range(NBLK):
            e0, e1 = int(bb[b]), int(bb[b + 1])
            n = e1 - e0
            o = blk_start[b] * P
            srcp[k, o:o + n] = srcs[e0:e1]
            dstlp[k, o:o + n] = (dsts[e0:e1] - (k * NPC + b * 128)).astype(BF)
            cmp_[k, o:o + n] = Cs[e0:e1]
            basp[k, :NG, o:o + n] = basis_s[e0:e1].T.astype(BF)
            basp[k, NG, o:o + n] = np.float32(1.0)

    def col(a):  # [EP] -> [P, TT] with [p,t] = a[t*P+p]
        return np.ascontiguousarray(a.reshape(TT, P).T)

    xT = np.ascontiguousarray(x.T).astype(BF)
    fw1T = np.concatenate([fw1.T, fb1[None, :]], axis=0).astype(BF)
    fw2T = np.ascontiguousarray(fw2.T).astype(BF)
    fb2two = _hilo(fb2)
    w1T = np.ascontiguousarray(w1.T).astype(BF)
    w2T = np.ascontiguousarray(w2.T).astype(BF)
    b2two = _hilo(b2)
    w3T = np.ascontiguousarray(w3.T).astype(BF)
    b3two = _hilo(b3)
    ones2 = np.ones((2, P), BF)
    iota = np.tile(np.arange(P, dtype=np.float32)[None, :], (P, 1)).astype(BF)

    nc = _build_nc(TT, blk_start, blk_end, block_of_tile)

    in_maps = []
    for k in range(NCORES):
        in_maps.append({
            "xT": xT, "basisT": np.ascontiguousarray(basp[k]),
            "srcI": col(srcp[k]), "dstl": col(dstlp[k]), "cmul": col(cmp_[k]),
            "fw1T": fw1T, "fw2T": fw2T, "fb2two": fb2two, "w1T": w1T,
            "w2T": w2T, "b2two": b2two, "w3T": w3T, "b3two": b3two,
            "ones2": ones2, "iota": np.ascontiguousarray(iota),
        })
    res = run_bass_kernel_spmd(nc, in_maps, core_ids=list(range(NCORES)))
    LAST_RESULT = res

    out = np.empty((N, HID), np.float32)
    for k in range(NCORES):
        out[k * NPC:(k + 1) * NPC, :] = res.results[k]["outT"].T
    return out



# revision 4
# speedup vs baseline: 2.5516x; 1.7975x over previous
"""SchNet InteractionBlock on 8 trn2 NeuronCores (Bass/Tile).

Strategy: sort edges by dst on host; core k owns nodes [k*6250,(k+1)*6250)
and exactly the edges targeting them -> no all-reduce needed.
Scatter-add is a one-hot selection-matrix matmul accumulated per
128-node block in PSUM. Filter MLP runs in feature-on-partition layout
with constant stationary weights. x@w1.T is precomputed per core into a
bf16 DRAM table and rows are fetched per edge tile via indirect DMA.

ssp(x) = softplus(x)-log2 is computed as relu(x) + p(min(e^x, e^-x))
with p a deg-2 minimax fit of log1p(u)-log2 on [0,1] (|err| < 8.2e-3),
using only Exp/Relu/Copy from the single `exp_and_friends` ACT table.
Biases are injected as K=2 rank-1 matmuls (hi/lo bf16 split) into PSUM.
"""

import numpy as np
import ml_dtypes

import concourse.bacc as bacc
import concourse.bass as bass
import concourse.mybir as mybir
import concourse.tile as tile
from concourse.bass_utils import run_bass_kernel_spmd

N = 50000
E = 600000
HID = 128
NF = 128
NG = 50
CUTOFF = 10.0
NCORES = 8
NPC = N // NCORES          # 6250 nodes per core
NBLK = (NPC + 127) // 128  # 49 blocks (last one has 106 nodes)
P = 128

BF16 = mybir.dt.bfloat16
F32 = mybir.dt.float32
I32 = mybir.dt.int32
AF = mybir.ActivationFunctionType
OP = mybir.AluOpType
LOG2 = float(np.log(2.0))
BF = ml_dtypes.bfloat16

# deg-2 minimax of log1p(u) on [0,1], with -log2 folded into C0
C2 = -0.22253306
C1 = 0.90520375
C0 = 0.00818788 - LOG2

LAST_RESULT = None  # BassKernelResults of the most recent run (for test harness)


def _hilo(v):
    hi = v.astype(BF)
    lo = (v - hi.astype(np.float32)).astype(BF)
    return np.ascontiguousarray(np.stack([hi, lo]))


def _build_nc(TT, blk_start, blk_end, block_of_tile):
    EP = TT * P
    nc = bacc.Bacc()

    xT_d = nc.dram_tensor("xT", [HID, N], BF16, kind="ExternalInput")
    basisT_d = nc.dram_tensor("basisT", [NG + 1, EP], BF16, kind="ExternalInput")
    srcI_d = nc.dram_tensor("srcI", [P, TT], I32, kind="ExternalInput")
    dstl_d = nc.dram_tensor("dstl", [P, TT], BF16, kind="ExternalInput")
    cmul_d = nc.dram_tensor("cmul", [P, TT], F32, kind="ExternalInput")
    fw1T_d = nc.dram_tensor("fw1T", [NG + 1, NF], BF16, kind="ExternalInput")
    fw2T_d = nc.dram_tensor("fw2T", [NF, NF], BF16, kind="ExternalInput")
    fb2two_d = nc.dram_tensor("fb2two", [2, NF], BF16, kind="ExternalInput")
    w1T_d = nc.dram_tensor("w1T", [HID, NF], BF16, kind="ExternalInput")
    w2T_d = nc.dram_tensor("w2T", [NF, HID], BF16, kind="ExternalInput")
    b2two_d = nc.dram_tensor("b2two", [2, HID], BF16, kind="ExternalInput")
    w3T_d = nc.dram_tensor("w3T", [HID, HID], BF16, kind="ExternalInput")
    b3two_d = nc.dram_tensor("b3two", [2, HID], BF16, kind="ExternalInput")
    ones2_d = nc.dram_tensor("ones2", [2, P], BF16, kind="ExternalInput")
    iota_d = nc.dram_tensor("iota", [P, P], BF16, kind="ExternalInput")
    outT_d = nc.dram_tensor("outT", [HID, NPC], F32, kind="ExternalOutput")

    with tile.TileContext(nc) as tc:
        with (
            tc.tile_pool(name="const", bufs=1) as cp,
            tc.tile_pool(name="dram", bufs=1, space="DRAM") as dp,
            tc.tile_pool(name="arr", bufs=1) as arp,
            tc.tile_pool(name="bchunk", bufs=2) as bp,
            tc.tile_pool(name="xtp", bufs=2) as xp,
            tc.tile_pool(name="work", bufs=3) as wp,
            tc.tile_pool(name="hsp", bufs=2) as hp,
            tc.tile_pool(name="psA", bufs=2, space="PSUM") as psA,
            tc.tile_pool(name="psB", bufs=2, space="PSUM") as psB,
            tc.tile_pool(name="psC", bufs=2, space="PSUM") as psC,
            tc.tile_pool(name="psD", bufs=2, space="PSUM") as psD,
        ):
            def cload(dram, shape, dtype):
                t = cp.tile(shape, dtype, tag=dram.name)
                nc.sync.dma_start(out=t[:], in_=dram[:])
                return t

            xh_d = dp.tile([N, NF], BF16, tag="xh_tab")
            fw1T = cload(fw1T_d, [NG + 1, NF], BF16)
            fw2T = cload(fw2T_d, [NF, NF], BF16)
            fb2two = cload(fb2two_d, [2, NF], BF16)
            w1T = cload(w1T_d, [HID, NF], BF16)
            w2T = cload(w2T_d, [NF, HID], BF16)
            b2two = cload(b2two_d, [2, HID], BF16)
            w3T = cload(w3T_d, [HID, HID], BF16)
            b3two = cload(b3two_d, [2, HID], BF16)
            ones2 = cload(ones2_d, [2, P], BF16)
            iota = cload(iota_d, [P, P], BF16)

            srcI = arp.tile([P, TT], I32, tag="srcI")
            nc.sync.dma_start(out=srcI[:], in_=srcI_d[:])
            dstl = arp.tile([P, TT], BF16, tag="dstl")
            nc.sync.dma_start(out=dstl[:], in_=dstl_d[:])
            cmul = arp.tile([P, TT], F32, tag="cmul")
            nc.sync.dma_start(out=cmul[:], in_=cmul_d[:])
            outT = arp.tile([HID, NPC], F32, tag="outT")

            # ---- phase 1: xh table = x @ w1.T, stored bf16 [N, NF] ----
            CH = 2048
            for c0 in range(0, N, CH):
                c1 = min(c0 + CH, N)
                w = c1 - c0
                xt = xp.tile([P, CH], BF16, tag="xt")
                nc.sync.dma_start(out=xt[:, :w], in_=xT_d[:, c0:c1])
                xo = xp.tile([P, CH], BF16, tag="xo")
                nt = (w + P - 1) // P
                for j in range(nt):
                    n0 = j * P
                    n1 = min(n0 + P, w)
                    m = n1 - n0
                    ps = psB.tile([P, P], F32, tag="psB")
                    nc.tensor.matmul(out=ps[:m, :], lhsT=xt[:, n0:n1],
                                     rhs=w1T[:], start=True, stop=True)
                    nc.vector.tensor_copy(out=xo[:m, j * P:j * P + P], in_=ps[:m, :])
                for j in range(nt):
                    n0 = j * P
                    n1 = min(n0 + P, w)
                    m = n1 - n0
                    nc.sync.dma_start(out=xh_d[c0 + n0:c0 + n1, :],
                                      in_=xo[:m, j * P:j * P + P])

            # ---- phase 2: edges ----
            BT = 64  # tiles per basis chunk
            bch = None
            hsT = None
            agg = None
            for t in range(TT):
                chn, s = divmod(t, BT)
                if s == 0:
                    bch = bp.tile([NG + 1, BT * P], BF16, tag="bch")
                    w = min(BT * P, EP - chn * BT * P)
                    nc.sync.dma_start(out=bch[:, :w],
                                      in_=basisT_d[:, chn * BT * P:chn * BT * P + w])
                if t % 4 == 0:
                    h1 = psA.tile([P, 512], F32, tag="h1")
                    nc.tensor.matmul(out=h1[:], lhsT=fw1T[:],
                                     rhs=bch[:, s * P:(s + 4) * P],
                                     start=True, stop=True)
                    # ssp(x) = relu(x) + p(min(e^x, e^-x))
                    e1 = hp.tile([P, 512], F32, tag="e1")
                    nc.scalar.activation(e1[:], h1[:], AF.Exp, scale=-1.0)
                    e2 = hp.tile([P, 512], F32, tag="e2")
                    nc.scalar.activation(e2[:], h1[:], AF.Exp, scale=1.0)
                    rl = hp.tile([P, 512], F32, tag="rl")
                    nc.scalar.activation(rl[:], h1[:], AF.Relu)
                    u = hp.tile([P, 512], F32, tag="u")
                    nc.vector.tensor_tensor(out=u[:], in0=e1[:], in1=e2[:], op=OP.min)
                    q = hp.tile([P, 512], F32, tag="q")
                    nc.vector.tensor_scalar(q[:], u[:], C2, C1, OP.mult, OP.add)
                    tq = hp.tile([P, 512], F32, tag="tq")
                    nc.vector.tensor_mul(out=tq[:], in0=q[:], in1=u[:])
                    hsT = hp.tile([P, 512], BF16, tag="hsT")
                    nc.vector.affine_then_add(hsT[:], tq[:], rl[:], 1.0, C0)
                q = t % 4
                wq = psB.tile([P, P], F32, tag="psB")
                nc.tensor.matmul(out=wq[:], lhsT=hsT[:, q * P:(q + 1) * P],
                                 rhs=fw2T[:], start=True, stop=False)
                nc.tensor.matmul(out=wq[:], lhsT=ones2[:], rhs=fb2two[:],
                                 start=False, stop=True, skip_group_check=True)
                xg = wp.tile([P, NF], BF16, tag="xg")
                nc.gpsimd.indirect_dma_start(
                    out=xg[:], out_offset=None, in_=xh_d[:],
                    in_offset=bass.IndirectOffsetOnAxis(ap=srcI[:, t:t + 1], axis=0),
                )
                # msg = (wq * C) * xh_gather
                msg = wp.tile([P, NF], BF16, tag="msg")
                nc.vector.scalar_tensor_tensor(out=msg[:], in0=wq[:],
                                               scalar=cmul[:, t:t + 1], in1=xg[:],
                                               op0=OP.mult, op1=OP.mult)
                S = wp.tile([P, P], BF16, tag="S")
                nc.vector.tensor_tensor(out=S[:],
                                        in0=dstl[:, t:t + 1].to_broadcast([P, P]),
                                        in1=iota[:], op=OP.is_equal)
                b = block_of_tile[t]
                if t == blk_start[b]:
                    agg = psC.tile([P, P], F32, tag="agg")
                nc.tensor.matmul(out=agg[:], lhsT=msg[:], rhs=S[:],
                                 start=(t == blk_start[b]), stop=(t == blk_end[b]),
                                 skip_group_check=True)
                if t == blk_end[b]:
                    nb = min(P, NPC - b * P)
                    aggs = wp.tile([P, P], BF16, tag="aggs")
                    nc.scalar.copy(out=aggs[:], in_=agg[:])
                    z1 = psD.tile([P, P], F32, tag="z")
                    nc.tensor.matmul(out=z1[:, :nb], lhsT=w2T[:],
                                     rhs=aggs[:, :nb], start=True, stop=False)
                    nc.tensor.matmul(out=z1[:, :nb], lhsT=b2two[:],
                                     rhs=ones2[:, :nb], start=False, stop=True,
                                     skip_group_check=True)
                    e1z = wp.tile([P, P], F32, tag="e1z")
                    nc.scalar.activation(e1z[:, :nb], z1[:, :nb], AF.Exp, scale=-1.0)
                    e2z = wp.tile([P, P], F32, tag="e2z")
                    nc.scalar.activation(e2z[:, :nb], z1[:, :nb], AF.Exp, scale=1.0)
                    rlz = wp.tile([P, P], F32, tag="rlz")
                    nc.scalar.activation(rlz[:, :nb], z1[:, :nb], AF.Relu)
                    uz = wp.tile([P, P], F32, tag="uz")
                    nc.vector.tensor_tensor(out=uz[:, :nb], in0=e1z[:, :nb],
                                            in1=e2z[:, :nb], op=OP.min)
                    qz = wp.tile([P, P], F32, tag="qz")
                    nc.vector.tensor_scalar(qz[:, :nb], uz[:, :nb], C2, C1,
                                            OP.mult, OP.add)
                    tqz = wp.tile([P, P], F32, tag="tqz")
                    nc.vector.tensor_mul(out=tqz[:, :nb], in0=qz[:, :nb],
                                         in1=uz[:, :nb])
                    z1s = wp.tile([P, P], BF16, tag="z1s")
                    nc.vector.affine_then_add(z1s[:, :nb], tqz[:, :nb], rlz[:, :nb],
                                              1.0, C0)
                    z2 = psD.tile([P, P], F32, tag="z")
                    nc.tensor.matmul(out=z2[:, :nb], lhsT=w3T[:],
                                     rhs=z1s[:, :nb], start=True, stop=False)
                    nc.tensor.matmul(out=z2[:, :nb], lhsT=b3two[:],
                                     rhs=ones2[:, :nb], start=False, stop=True,
                                     skip_group_check=True)
                    nc.scalar.copy(out=outT[:, b * P:b * P + nb], in_=z2[:, :nb])

            nc.sync.dma_start(out=outT_d[:], in_=outT[:])

    nc.compile()
    return nc


def build_for_sim(**inputs):
    """Build the Bass module for core 0 without executing (local sim use)."""
    ji = np.asarray(inputs["ji_pairs"])
    dst = ji[1].astype(np.int64)
    order = np.argsort(dst, kind="stable")
    dsts = dst[order]
    blk_bounds = []
    for k in range(NCORES):
        marks = k * NPC + np.minimum(np.arange(NBLK + 1) * 128, NPC)
        blk_bounds.append(np.searchsorted(dsts, marks))
    cnt = np.array([bb[1:] - bb[:-1] for bb in blk_bounds])
    T = np.maximum(1, -(-cnt // P)).max(axis=0)
    if T.sum() % 4:
        T[-1] += 4 - T.sum() % 4
    TT = int(T.sum())
    tile_ofs = np.concatenate([[0], np.cumsum(T)])
    blk_start = [int(tile_ofs[b]) for b in range(NBLK)]
    blk_end = [int(tile_ofs[b + 1] - 1) for b in range(NBLK)]
    block_of_tile = np.repeat(np.arange(NBLK), T)
    return _build_nc(TT, blk_start, blk_end, block_of_tile)


def prepare(**inputs):
    x = np.asarray(inputs["x"], np.float32)
    ji = np.asarray(inputs["ji_pairs"])
    e_ji = np.asarray(inputs["e_ji"], np.float32)
    basis = np.asarray(inputs["e_ji_basis"], np.float32)
    fw1 = np.asarray(inputs["fw1"], np.float32)
    fb1 = np.asarray(inputs["fb1"], np.float32)
    fw2 = np.asarray(inputs["fw2"], np.float32)
    fb2 = np.asarray(inputs["fb2"], np.float32)
    w1 = np.asarray(inputs["w1"], np.float32)
    w2 = np.asarray(inputs["w2"], np.float32)
    b2 = np.asarray(inputs["b2"], np.float32)
    w3 = np.asarray(inputs["w3"], np.float32)
    b3 = np.asarray(inputs["b3"], np.float32)

    src = ji[0].astype(np.int64)
    dst = ji[1].astype(np.int64)
    order = np.argsort(dst, kind="stable")
    dsts = dst[order]
    srcs = src[order].astype(np.int32)
    Cs = (0.25 * (np.cos(e_ji * (np.pi / CUTOFF)) + 1.0)).astype(np.float32)[order]
    basis_s = basis[order]

    # per (core, block) edge ranges
    blk_bounds = []
    for k in range(NCORES):
        marks = k * NPC + np.minimum(np.arange(NBLK + 1) * 128, NPC)
        blk_bounds.append(np.searchsorted(dsts, marks))
    cnt = np.array([bb[1:] - bb[:-1] for bb in blk_bounds])  # [NCORES, NBLK]
    T = np.maximum(1, -(-cnt // P)).max(axis=0)              # tiles per block
    if T.sum() % 4:
        T[-1] += 4 - T.sum() % 4
    TT = int(T.sum())
    EP = TT * P
    tile_ofs = np.concatenate([[0], np.cumsum(T)])
    blk_start = [int(tile_ofs[b]) for b in range(NBLK)]
    blk_end = [int(tile_ofs[b + 1] - 1) for b in range(NBLK)]
    block_of_tile = np.repeat(np.arange(NBLK), T)

    srcp = np.zeros((NCORES, EP), np.int32)
    dstlp = np.full((NCORES, EP), -1.0, BF)
    cmp_ = np.zeros((NCORES, EP), np.float32)
    basp = np.zeros((NCORES, NG + 1, EP), BF)
    for k in range(NCORES):
        bb = blk_bounds[k]
        for b in range(NBLK):
            e0, e1 = int(bb[b]), int(bb[b + 1])
            n = e1 - e0
            o = blk_start[b] * P
            srcp[k, o:o + n] = srcs[e0:e1]
            dstlp[k, o:o + n] = (dsts[e0:e1] - (k * NPC + b * 128)).astype(BF)
            cmp_[k, o:o + n] = Cs[e0:e1]
            basp[k, :NG, o:o + n] = basis_s[e0:e1].T.astype(BF)
            basp[k, NG, o:o + n] = np.float32(1.0)

    def col(a):  # [EP] -> [P, TT] with [p,t] = a[t*P+p]
        return np.ascontiguousarray(a.reshape(TT, P).T)

    xT = np.ascontiguousarray(x.T).astype(BF)
    fw1T = np.concatenate([fw1.T, fb1[None, :]], axis=0).astype(BF)
    fw2T = np.ascontiguousarray(fw2.T).astype(BF)
    fb2two = _hilo(fb2)
    w1T = np.ascontiguousarray(w1.T).astype(BF)
    w2T = np.ascontiguousarray(w2.T).astype(BF)
    b2two = _hilo(b2)
    w3T = np.ascontiguousarray(w3.T).astype(BF)
    b3two = _hilo(b3)
    ones2 = np.ones((2, P), BF)
    iota = np.tile(np.arange(P, dtype=np.float32)[None, :], (P, 1)).astype(BF)

    nc = _build_nc(TT, blk_start, blk_end, block_of_tile)

    in_maps = []
    for k in range(NCORES):
        in_maps.append({
            "xT": xT, "basisT": np.ascontiguousarray(basp[k]),
            "srcI": col(srcp[k]), "dstl": col(dstlp[k]), "cmul": col(cmp_[k]),
            "fw1T": fw1T, "fw2T": fw2T, "fb2two": fb2two, "w1T": w1T,
            "w2T": w2T, "b2two": b2two, "w3T": w3T, "b3two": b3two,
            "ones2": ones2, "iota": np.ascontiguousarray(iota),
        })
    return nc, in_maps


def kernel(**inputs):
    global LAST_RESULT
    nc, in_maps = prepare(**inputs)
    res = run_bass_kernel_spmd(nc, in_maps, core_ids=list(range(NCORES)))
    LAST_RESULT = res

    out = np.empty((N, HID), np.float32)
    for k in range(NCORES):
        out[k * NPC:(k + 1) * NPC, :] = res.results[k]["outT"].T
    return out

